# revision 28
# baseline (speedup 1.0000x reference)
"""Trainium2 Bass kernel for nn_AltAttention (dense transformer attention block).

Reference computation (B=4, S=2048, D=512, H=8, Dh=64):
    qkv  = hidden @ W_qkv + b_qkv                      -> q, k, v per head
    attn = softmax(q k^T * D**-0.5 + alibi, masked)
    out  = (attn @ v) @ W_proj + b_proj
Sharding: one head per NeuronCore (8 heads = 8 cores), partial proj outputs
summed on the host (the tensor-parallel all-reduce).

Structure (per core):
  phase 1: q|k and v projections from the full hidden states. q and k land in
    one combined SBUF tile (q rows 0:64, k rows 64:128) so the qkv bias add is
    a single 128-partition DVE op; the k half is then copied to partition base
    0 with one SBUF DMA per batch (matmul operands must share a base
    partition). v^T tiles are DMA-transposed into the attn@V operand with a
    ones column at col 63 (softmax denominator rides row 0 of attn@V output).
    The v bias is folded into wproj's bias row on the host (softmax rows sum
    to 1, so it contributes exactly bv @ W_proj_head).
  phase 2 per (batch, 512-query block):
    - scores: 16 K=64 matmuls [keys=128, 512] (transposed score tiles)
    - exp on ACT (exp(s+alibi) = exp(s) * precomputed exp(alibi)); the
      exp(alibi) multiply alternates DVE / GPSIMD to balance engine load
    - attn@V accumulates [65, 512] (row 0 = softmax denominators)
    - normalization is applied AFTER the proj matmul: per m-block, a tiny
      N=1 matmul (sharing the proj lhsT) extracts the denominators
      transposed (queries into partitions), a [128,1] DVE reciprocal, and
      the PSUM->SBUF eviction becomes a tensor_scalar multiply by 1/sum.
      The bias row of W_proj_aug is multiplied by the denominator in the
      proj matmul, so the scaling restores exactly the bias (only core 0
      carries b_proj).
  output DMA is contiguous per partition ([B, NQB, 128, 4*512] bf16 layout),
  un-permuted and summed across cores on the host.
"""

import sys

sys.path.insert(0, "/opt/trn_rl_repo")

import numpy as np
import ml_dtypes

import concourse.bass as bass
import concourse.tile as tile
from concourse import bacc, mybir
from concourse.bass_utils import run_bass_kernel_spmd

BF16 = mybir.dt.bfloat16
F32 = mybir.dt.float32
NP_BF16 = ml_dtypes.bfloat16

B, S, D, H = 4, 2048, 512, 8
Dh = D // H  # 64
BS = B * S  # 8192
P = 128
NKT = S // P  # 16 ks tiles per batch
NQB = S // 512  # 4 query blocks of 512 per batch
NSC = S // 512  # 4 s-chunks of 512 per batch (qkv phase)
SCALE = D ** (-0.5)
POOL_GROUPS = (0, 2, 4, 6)  # exp(alibi) multiplies routed to GPSIMD (of 8)


def build_program(eb: int, repeat: int = 1, phases=(1, 2), skel=False):
    """Build the per-core Bass program. eb = number of exp-alibi slices
    (1 when the attention mask is all ones, B otherwise)."""
    nc = bacc.Bacc("TRN2", target_bir_lowering=False, debug=False, num_devices=H)

    hiddenT = nc.dram_tensor("hiddenT", [D, BS], BF16, kind="ExternalInput")
    # ea layout: [eb, NQB, 128, NKT, 512] so each (e, qb) slice is one
    # contiguous 2 MB DMA
    ea = nc.dram_tensor("ea", [eb, NQB, P, NKT, 512], BF16,
                        kind="ExternalInput")
    wqk = nc.dram_tensor("wqk", [4, P, P], BF16, kind="ExternalInput")
    bqk = nc.dram_tensor("bqk", [P, 1], F32, kind="ExternalInput")
    wv = nc.dram_tensor("wv", [4, P, Dh], BF16, kind="ExternalInput")
    wproj = nc.dram_tensor("wproj", [Dh + 1, D], BF16, kind="ExternalInput")
    part = nc.dram_tensor("part", [B, NQB, P, 4, 512], BF16,
                          kind="ExternalOutput")

    hT_re = hiddenT[:].rearrange("(c p) s -> p c s", p=P)  # [128, 4, 8192]

    with tile.TileContext(nc) as tc:
        with tc.tile_pool(name="consts", bufs=1) as consts, \
             tc.tile_pool(name="persist", bufs=1) as persist:
            wqk_sb = consts.tile([P, 4, P], BF16)
            nc.sync.dma_start(wqk_sb[:], wqk[:].rearrange("c p m -> p c m"))
            wv_sb = consts.tile([P, 4, Dh], BF16)
            nc.sync.dma_start(wv_sb[:], wv[:].rearrange("c p m -> p c m"))
            bqk_sb = consts.tile([P, 1], F32)
            nc.sync.dma_start(bqk_sb[:], bqk[:])
            wproj_sb = consts.tile([Dh + 1, D], BF16)
            nc.sync.dma_start(wproj_sb[:], wproj[:])
            # e0 column: 1 in row 0, 0 elsewhere — extracts the denominator
            # row of xs_t transposed (queries into partitions)
            e0_col = consts.tile([Dh + 1, 1], BF16)
            nc.vector.memset(e0_col[:], 0.0)
            nc.vector.memset(e0_col[0:1, :], 1.0)

            qkT = persist.tile([P, BS], BF16)  # rows 0:64 q^T, 64:128 k^T
            kT = persist.tile([Dh, BS], BF16)  # k^T at partition base 0
            # attn@V operand, tile t = [:, t, 63:128]: partitions = keys of
            # tile t, col 63 = ones (sums row), cols 64:128 = v dims
            # (DMA-transpose needs 128B-aligned dest offsets, hence col 63)
            vaug = persist.tile([P, B * NKT, P], BF16)
            nc.vector.memset(vaug[:, :, Dh - 1 : Dh], 1.0)
            vt_all = persist.tile([Dh, B * NSC, 512], BF16)

            if skel:
                p_fix = persist.tile([P, NKT, 512], BF16)
                nc.vector.memset(p_fix[:], 0.01)
                xs_fix = persist.tile([Dh + 1, 512], BF16)
                nc.vector.memset(xs_fix[:], 0.02)
                srec_fix = persist.tile([P, 4], F32)
                nc.vector.memset(srec_fix[:], 1.0)
            if 1 not in phases:
                nc.vector.memset(qkT[:], 0.01)
                nc.vector.memset(kT[:], 0.01)
                nc.vector.memset(vaug[:], 0.01)
                nc.vector.memset(vaug[:, :, Dh - 1 : Dh], 1.0)

            for rep in range(repeat):
             with tc.tile_pool(name="eapool", bufs=2) as eapool:
                ea_pre = None
                # ---------------- phase 1: qkv projections ----------------
                if 1 in phases:
                 with tc.tile_pool(name="hpool", bufs=4) as hpool, \
                      tc.tile_pool(name="qkps", bufs=2, space="PSUM") as qkps, \
                      tc.tile_pool(name="vtps", bufs=2, space="PSUM") as vtps:
                    # issue all hidden-state loads up front so the sync DMA
                    # queue streams them back-to-back (nothing FIFOs ahead
                    # of a load the PE is about to need)
                    hts = []
                    for b in range(B):
                        ht = hpool.tile([P, 4, S], BF16)
                        nc.sync.dma_start(ht[:],
                                          hT_re[:, :, b * S : (b + 1) * S])
                        hts.append(ht)
                    for b in range(B):
                        ht = hts[b]
                        for sci in range(NSC):
                            col0 = b * S + sci * 512
                            ssl = slice(sci * 512, (sci + 1) * 512)

                            qk_ps = qkps.tile([P, 512], F32)
                            for c in range(4):
                                nc.tensor.matmul(qk_ps[:], wqk_sb[:, c, :],
                                                 ht[:, c, ssl],
                                                 start=(c == 0), stop=(c == 3))
                            sl = slice(col0, col0 + 512)
                            nc.vector.tensor_scalar_add(
                                qkT[:, sl], qk_ps[:], bqk_sb[:])

                            # v^T tiles; the v bias is folded into wproj
                            # row 0 on the host (softmax rows sum to 1)
                            vt_ps = vtps.tile([Dh, 512], F32)
                            for c in range(4):
                                nc.tensor.matmul(vt_ps[:], wv_sb[:, c, :],
                                                 ht[:, c, ssl],
                                                 start=(c == 0), stop=(c == 3))
                            i = b * NSC + sci
                            nc.vector.tensor_copy(vt_all[:, i, :], vt_ps[:])
                        bsl = slice(b * S, (b + 1) * S)
                        # k^T copy on the ACT HWDGE ring: runs in parallel
                        # with the hidden-state loads on the sync ring
                        nc.scalar.dma_start(kT[:, bsl], qkT[Dh:P, bsl])
                        if b == B - 1 and 2 in phases:
                            # prefetch the first exp-alibi tile during the
                            # last qkv batch so phase 2 starts without a
                            # 2 MB DMA stall (also on the ACT ring)
                            ea_pre = eapool.tile([P, NKT, 512], BF16)
                            nc.scalar.dma_start(ea_pre[:], ea[0, 0])
                    # all transposes back-to-back: a single XBAR-mode
                    # transition on the DMA path instead of one per chunk
                    for i in range(B * NSC):
                        nc.sync.dma_start(vaug[:, i * 4 : i * 4 + 4, Dh:P],
                                          vt_all[:, i, :], transpose=True)

                # ---------------- phase 2: attention + proj ----------------
                if 2 in phases:
                 with tc.tile_pool(name="ppool", bufs=2) as ppool, \
                      tc.tile_pool(name="xspool", bufs=2) as xspool, \
                      tc.tile_pool(name="recpool", bufs=2) as recpool, \
                      tc.tile_pool(name="outpool", bufs=2) as outpool, \
                      tc.tile_pool(name="spool", bufs=2, space="PSUM") as spool, \
                      tc.tile_pool(name="xpool", bufs=1, space="PSUM") as xpool, \
                      tc.tile_pool(name="s4pool", bufs=1, space="PSUM") as s4pool, \
                      tc.tile_pool(name="ops", bufs=2, space="PSUM") as ops:
                    ea_next = ea_pre
                    for qb in range(NQB):
                        if eb == 1:
                            if ea_next is not None:
                                ea_t = ea_next
                                ea_next = None
                            else:
                                ea_t = eapool.tile([P, NKT, 512], BF16)
                                nc.sync.dma_start(ea_t[:], ea[0, qb])
                        for b in range(B):
                            if eb != 1:
                                if qb == 0 and b == 0 and ea_pre is not None:
                                    ea_t = ea_pre
                                else:
                                    ea_t = eapool.tile([P, NKT, 512], BF16)
                                    nc.sync.dma_start(ea_t[:], ea[b, qb])
                            elif b == 1 and qb + 1 < NQB:
                                # prefetch next query block's exp-alibi tile
                                # mid-loop so the qb boundary has no DMA stall
                                ea_next = eapool.tile([P, NKT, 512], BF16)
                                nc.sync.dma_start(ea_next[:], ea[0, qb + 1])
                            qsl = slice(b * S + qb * 512, b * S + (qb + 1) * 512)
                            x_ps = xpool.tile([Dh + 1, 512], F32)
                            p_all = ppool.tile([P, NKT, 512], BF16)
                            for g in range(NKT // 2):
                                s_ps = spool.tile([P, 1024], F32)
                                for j in range(2):
                                    tk = g * 2 + j
                                    ksl = slice(b * S + tk * P,
                                                b * S + (tk + 1) * P)
                                    nc.tensor.matmul(
                                        s_ps[:, j * 512 : (j + 1) * 512],
                                        kT[:, ksl], qkT[0:Dh, qsl],
                                        start=True, stop=True)
                                psl = p_all[:, 2 * g : 2 * g + 2, :].rearrange(
                                    "p a b -> p (a b)")
                                easl = ea_t[:, 2 * g : 2 * g + 2, :].rearrange(
                                    "p a b -> p (a b)")
                                if not skel:
                                    nc.scalar.activation(
                                        psl, s_ps[:],
                                        mybir.ActivationFunctionType.Exp)
                                    mul_eng = (nc.gpsimd if g in POOL_GROUPS
                                               else nc.vector)
                                    mul_eng.tensor_mul(psl, psl, easl)
                                for j in range(2):
                                    tk = g * 2 + j
                                    t = b * NKT + tk
                                    nc.tensor.matmul(
                                        x_ps[:], vaug[:, t, Dh - 1 : P],
                                        (p_fix if skel else p_all)[:, tk, :],
                                        start=(tk == 0), stop=(tk == NKT - 1))
                            if skel:
                                xs_t = xs_fix
                                s_rec = srec_fix
                            else:
                             xs_t = xspool.tile([Dh + 1, 512], BF16)
                             nc.vector.tensor_copy(xs_t[:], x_ps[:])
                             s4 = s4pool.tile([P, 4], F32, tag="s4")
                             s_rec = recpool.tile([P, 4], F32)
                            out_sb = outpool.tile([P, 4, 512], BF16)
                            for m in range(4):
                                msl = slice(m * P, (m + 1) * P)
                                if not skel:
                                    # denominators transposed (queries into
                                    # partitions); same lhsT as the proj
                                    # matmul that follows
                                    nc.tensor.matmul(
                                        s4[:, m : m + 1], xs_t[:, msl],
                                        e0_col[:], start=True, stop=True)
                                out_ps = ops.tile([P, 512], F32, tag="ops")
                                nc.tensor.matmul(out_ps[:], xs_t[:, msl],
                                                 wproj_sb[:],
                                                 start=True, stop=True)
                                if not skel:
                                    nc.vector.reciprocal(
                                        s_rec[:, m : m + 1],
                                        s4[:, m : m + 1])
                                nc.vector.tensor_scalar_mul(
                                    out_sb[:, m, :], out_ps[:],
                                    s_rec[:, m : m + 1])
                            nc.sync.dma_start(part[b, qb], out_sb[:])

    nc.compile()
    return nc


_CACHE = {}


def _get_program(eb: int):
    key = ("prog", eb)
    if key not in _CACHE:
        _CACHE[key] = build_program(eb)
    return _CACHE[key]


def prepare_inputs(hidden_states, attention_mask, alibi_bias, W_qkv, b_qkv,
                   W_proj, b_proj):
    """Host-side prep: transposes, scale folding, exp(alibi), bf16 casts.
    Returns (in_maps, eb)."""
    hidden_states = np.asarray(hidden_states, dtype=np.float32)
    attention_mask = np.asarray(attention_mask)
    alibi_bias = np.asarray(alibi_bias, dtype=np.float32)
    W_qkv = np.asarray(W_qkv, dtype=np.float32)
    b_qkv = np.asarray(b_qkv, dtype=np.float32)
    W_proj = np.asarray(W_proj, dtype=np.float32)
    b_proj = np.asarray(b_proj, dtype=np.float32)

    # split the score scale evenly between the q and k operands
    s_side = np.float32(np.sqrt(SCALE))

    hiddenT = np.ascontiguousarray(
        hidden_states.reshape(BS, D).T).astype(NP_BF16)

    mask_trivial = bool(attention_mask.all())
    eb = 1 if mask_trivial else B

    def ea_layout(eaT):
        # eaT [S(k), S(q)] -> [NQB, 128, NKT, 512] contiguous per qb slice
        return np.ascontiguousarray(
            eaT.reshape(NKT, P, NQB, 512).transpose(2, 1, 0, 3))

    ea_all = []
    for h in range(H):
        eaT = np.exp(alibi_bias[0, h].T).astype(NP_BF16)  # [S(k), S(q)]
        if mask_trivial:
            ea_all.append(ea_layout(eaT)[None])
        else:
            me = np.where(attention_mask, 1.0, 0.0).astype(NP_BF16)  # [B, S]
            ea_all.append(np.stack(
                [ea_layout(eaT * me[bi][:, None]) for bi in range(B)]))
    in_maps = []
    for h in range(H):
        # reference reshapes qkv to (B, S, H, 3*Dh) then splits: head h's
        # q/k/v live in columns [h*3*Dh, h*3*Dh + 3*Dh)
        qs = slice(h * 3 * Dh, h * 3 * Dh + Dh)
        ks = slice(h * 3 * Dh + Dh, h * 3 * Dh + 2 * Dh)
        vs = slice(h * 3 * Dh + 2 * Dh, h * 3 * Dh + 3 * Dh)
        wqk_h = np.concatenate([W_qkv[:, qs], W_qkv[:, ks]], axis=1) * s_side
        bqk_h = np.concatenate([b_qkv[qs], b_qkv[ks]]) * s_side
        wv_h = W_qkv[:, vs]
        bv_h = b_qkv[vs]
        # v bias folded into the wproj bias row: softmax rows sum to 1, so
        # the v bias contributes exactly bv @ W_proj_head to the output
        wp_h = W_proj[h * Dh : (h + 1) * Dh, :]
        bias_row = bv_h @ wp_h + (b_proj if h == 0 else 0.0)
        wproj_aug = np.concatenate([bias_row[None, :], wp_h], axis=0)
        in_maps.append({
            "hiddenT": hiddenT,
            "ea": ea_all[h],
            "wqk": np.ascontiguousarray(
                wqk_h.reshape(4, P, P).astype(NP_BF16)),
            "bqk": np.ascontiguousarray(bqk_h[:, None]),
            "wv": np.ascontiguousarray(wv_h.reshape(4, P, Dh).astype(NP_BF16)),
            "wproj": wproj_aug.astype(NP_BF16),
        })
    return in_maps, eb


def kernel(**inputs):
    in_maps, eb = prepare_inputs(**inputs)
    nc = _get_program(eb)
    res = run_bass_kernel_spmd(nc, in_maps, list(range(H)))
    out = res.results[0]["part"].astype(np.float32)
    for h in range(1, H):
        out = out + res.results[h]["part"]
    # [B, NQB, P, 4, 512] -> [B, S, D]: row within a 512-query block is
    # m*128 + p
    return np.ascontiguousarray(
        out.transpose(0, 1, 3, 2, 4).reshape(B, S, D))
